# revision 28
# baseline (speedup 1.0000x reference)
"""GP prediction kernel for Trainium2 (8 NeuronCores, data-parallel over batch).

Computes z_pred[b, p, d] = sum_c k_mult[p, c] * z_enc[b, c, d] where k_mult
is the [64, 448] GP weight matrix k_pred.T @ inv(cov + sigma*I). k_mult
depends only on compile-time constants, so it is precomputed on host; the
device work is a batched [64,448] @ [448,1024] matmul, sharded 8 batches
per core.

Precision: the correctness gate is rel_err < 2e-2. k_mult's energy is
concentrated in the recent context columns, so the far 384 context rows
move as fp8e4m3 (1 B) and the near 64 rows as fp16; k, the near-z and the
output stay fp16. The PE accepts mixed fp16-lhsT x fp8-rhs matmuls
directly (probed: exact), so no on-chip casts are needed. Measured
rel_err ~1.2e-2 on the reference inputs (deterministic seed).

DMA: contraction tiled as 448 = 3*128 (fp8) + 64 (fp16). Per batch one
fp8 load ([128, 3 KB] host-swizzled contiguous, full 16-SDMA-engine
spread) + one small fp16 load, interleaved across the two HWDGE queues
(sync=SP, scalar=ACT); output stores per 512-column half ride the gpsimd
SWDGE queue except the last pair's, which use the HWDGE queues. A short
dummy-matmul warmup runs while the first loads stream so HAM un-throttles
the PE clock (1.2 -> 2.4 GHz) before the real matmuls begin.

PE: batch pairs are column-tiled - batch 2bp in PE columns 0-63, batch
2bp+1 in columns 64-127, accumulating into one [128, 512] PSUM bank - so
paired matmuls run concurrently on the array. n (column half) is the
outer loop so the first half's PSUM->SBUF copy and store overlap the
second half's matmuls.
"""
import numpy as np
import ml_dtypes
from contextlib import ExitStack

import concourse.bacc as bacc
import concourse.tile as tile
from concourse import mybir
from concourse.bass_utils import run_bass_kernel_spmd

# Problem constants (hardcoded per harness contract).
B, T, D = 64, 512, 1024
P = 64                 # N_PREDICTORS
C = T - P              # 448 context timesteps
L, SIGMA, TIMESCALE = 0.01, 0.01, 0.3
N_CORES = 8
BPC = B // N_CORES     # batches per core

NJ8 = 3                # fp8 K-tiles of 128 rows (far context)
C8 = NJ8 * 128         # 384
KN = C - C8            # 64 fp16 rows (near context)
NJ = 4                 # total K-tiles per batch


def _k_mult_T() -> np.ndarray:
    """[C, P] transpose of the GP weight matrix.

    Replicates the reference's fp32 jax ops on CPU so the constant matches
    the reference's k_mult near-bitwise; falls back to a float64 numpy solve.
    """
    try:
        import jax
        import jax.numpy as jnp

        cpu = jax.devices("cpu")[0]
        with jax.default_device(cpu):
            t = jnp.linspace(0.0, 1.0, T)
            t_in = t[:C] * TIMESCALE
            t_pred = t[C:] * TIMESCALE

            def rbf(x, y):
                d = x[:, None] - y[None, :]
                return jnp.exp(-0.5 * (d * d) / L)

            cov = rbf(t_in, t_in)
            k_pred = rbf(t_in, t_pred)
            eye = jnp.eye(C, dtype=cov.dtype)
            k_mult = k_pred.T @ jnp.linalg.inv(cov + eye * SIGMA)   # [P, C]
            km_T = np.asarray(k_mult).T                             # [C, P]
    except Exception:
        t = np.linspace(0.0, 1.0, T)
        t_in = t[:C] * TIMESCALE
        t_pred = t[C:] * TIMESCALE

        def rbf_np(x, y):
            d = x[:, None] - y[None, :]
            return np.exp(-0.5 * d * d / L)

        cov = rbf_np(t_in, t_in) + np.eye(C) * SIGMA
        km_T = np.linalg.solve(cov, rbf_np(t_in, t_pred))
    return np.ascontiguousarray(km_T.astype(np.float32))


def _km_packed() -> np.ndarray:
    """[128, NJ*P] fp16: column block j holds K-tile j of k_mult.T.
    The 64-row near tile (j=3) is duplicated into rows 64-127 so odd
    batches of a pair can matmul it from SBUF partitions 64-127."""
    km_T = _k_mult_T().astype(np.float16)      # [C, P]
    out = np.zeros((128, NJ * P), np.float16)
    for j in range(NJ8):
        out[:, j * P : (j + 1) * P] = km_T[j * 128 : (j + 1) * 128]
    out[:KN, NJ8 * P : NJ * P] = km_T[C8:]
    out[KN:, NJ8 * P : NJ * P] = km_T[C8:]
    return np.ascontiguousarray(out)


KM_PACKED = _km_packed()

_NC = None


def _build():
    nc = bacc.Bacc()
    # z8 arrives host-swizzled: row b*128+p holds the three far chunks of
    # batch b, partition p — 3 KB contiguous per partition.
    z8 = nc.dram_tensor("z8", [BPC * 128, NJ8 * D], mybir.dt.float8e4,
                        kind="ExternalInput")
    # z16: near rows of a whole batch PAIR per 128-partition row block
    # (batch 2bp in partitions 0-63, batch 2bp+1 in 64-127).
    z16 = nc.dram_tensor("z16", [(BPC // 2) * 128, D], mybir.dt.float16,
                         kind="ExternalInput")
    km = nc.dram_tensor("km", [128, NJ * P], mybir.dt.float16,
                        kind="ExternalInput")
    out = nc.dram_tensor("out", [BPC * P, D], mybir.dt.float16,
                         kind="ExternalOutput")

    with tile.TileContext(nc) as tc, ExitStack() as ctx:
        kpool = ctx.enter_context(tc.tile_pool(name="km", bufs=1))
        z8pool = ctx.enter_context(tc.tile_pool(name="z8", bufs=BPC))
        z16pool = ctx.enter_context(tc.tile_pool(name="z16", bufs=BPC))
        opool = ctx.enter_context(tc.tile_pool(name="o", bufs=4))
        ppool = ctx.enter_context(tc.tile_pool(name="ps", bufs=4, space="PSUM"))
        wpool = ctx.enter_context(tc.tile_pool(name="wm", bufs=1, space="PSUM"))

        km_sb = kpool.tile([128, NJ * P], mybir.dt.float16)
        nc.sync.dma_start(km_sb[:, :], km[:, :])

        # PE warmup while the first loads stream: dummy matmuls on the
        # weight tile un-throttle the HAM clock gate (1.2 -> 2.4 GHz)
        # before the real matmuls begin.
        warm_ps = wpool.tile([P, 256], mybir.dt.float32)
        for w in range(16):
            nc.tensor.matmul(
                warm_ps[:, :], km_sb[:, 0:P], km_sb[:, 0:256],
                start=True, stop=True,
            )

        # Phase 1: per batch one 384 KB fp8 load; per batch pair one
        # full-width 256 KB fp16 near load.
        zt8, zt16 = {}, {}
        for b in range(BPC):
            zt8[b] = z8pool.tile([128, NJ8 * D], mybir.dt.float8e4,
                                 name=f"z8t{b}", tag="z8t")
            eng = nc.sync if b % 2 == 0 else nc.scalar
            eng.dma_start(zt8[b][:, :], z8[b * 128 : (b + 1) * 128, :])
            if b % 2 == 1:
                bp = b // 2
                zt16[bp] = z16pool.tile([128, D], mybir.dt.float16,
                                        name=f"z16t{bp}", tag="z16t")
                eng = nc.sync if bp % 2 == 0 else nc.scalar
                eng.dma_start(zt16[bp][:, :],
                              z16[bp * 128 : (bp + 1) * 128, :])

        # Phase 2: column-tiled batch pairs, n outermost so the first
        # half's copy/store overlaps the second half's matmuls.
        for bp in range(BPC // 2):
            out_sb = opool.tile([128, D], mybir.dt.float16, name=f"osb{bp}",
                                tag="osb")
            for n in range(2):
                ps = ppool.tile([128, 512], mybir.dt.float32,
                                name=f"ps{bp}_{n}", tag="ps")
                for j in range(NJ):
                    for half in range(2):
                        b = 2 * bp + half
                        if j < NJ8:
                            lhsT = km_sb[:, j * P : (j + 1) * P]
                            rhs = zt8[b][:, j * D + n * 512
                                         : j * D + n * 512 + 512]
                            tp = (0, half * P)
                        else:
                            # near tile: even batch lives in partitions
                            # 64-127, odd in 0-63, so tile_position is
                            # (64,0) or (0,64) — never the broken (64,64)
                            # PE quadrant.
                            ko = (1 - half) * KN
                            lhsT = km_sb[ko : ko + KN, j * P : (j + 1) * P]
                            rhs = zt16[bp][ko : ko + KN,
                                           n * 512 : n * 512 + 512]
                            tp = (ko, half * P)
                        nc.tensor.matmul(
                            ps[half * P : (half + 1) * P, :],
                            lhsT,
                            rhs,
                            start=(j == 0), stop=(j == NJ - 1),
                            tile_position=tp,
                        )
                dst = out_sb[:, n * 512 : (n + 1) * 512]
                if bp == BPC // 2 - 1 and n == 1:
                    # last pair: run the two copies on different engines so
                    # they overlap, shortening the kernel tail
                    nc.scalar.activation(
                        dst, ps[:, :], mybir.ActivationFunctionType.Copy
                    )
                else:
                    nc.vector.tensor_copy(dst, ps[:, :])
                if bp < 3:
                    oeng = nc.gpsimd
                else:
                    oeng = nc.sync if n == 0 else nc.scalar
                oeng.dma_start(
                    out[bp * 128 : (bp + 1) * 128, n * 512 : (n + 1) * 512],
                    dst,
                )

    nc.finalize()
    return nc


def kernel(z_enc: np.ndarray, _trace: bool = False):
    global _NC
    z_enc = np.asarray(z_enc, dtype=np.float32)
    if _NC is None:
        _NC = _build()

    z8 = z_enc[:, :C8, :].astype(ml_dtypes.float8_e4m3)
    z16 = z_enc[:, C8:C, :].astype(np.float16)
    in_maps = []
    for i in range(N_CORES):
        sh8 = z8[i * BPC : (i + 1) * BPC]               # [BPC, C8, D]
        sw8 = np.ascontiguousarray(
            sh8.reshape(BPC, NJ8, 128, D).transpose(0, 2, 1, 3)
        ).reshape(BPC * 128, NJ8 * D)                   # [b*128+p, c*D+d]
        # near pairs: odd batch in partitions 0-63, even batch in 64-127
        sh16 = z16[i * BPC : (i + 1) * BPC].reshape(BPC // 2, 2, KN, D)
        sw16 = np.ascontiguousarray(sh16[:, ::-1]).reshape(
            (BPC // 2) * 128, D
        )
        in_maps.append({
            "z8": sw8,
            "z16": sw16,
            "km": KM_PACKED,
        })

    res = run_bass_kernel_spmd(_NC, in_maps, core_ids=list(range(N_CORES)),
                               trace=_trace)
    out = np.concatenate(
        [r["out"].astype(np.float32).reshape(BPC, P, D) for r in res.results],
        axis=0,
    )
    if _trace:
        return out, res
    return out
